# revision 34
# baseline (speedup 1.0000x reference)
"""Causal multi-head attention (B=4, S=2048, H=16, D=64, E=1024) on 8 TRN2 cores.

Sharding: data-parallel over batch (4) x tensor-parallel over heads (2 groups
of 8). Each core computes, for its (batch, head-group):
    q/k/v projections -> causal softmax attention -> output projection
and returns a partial [S, E] output (Wr row-split); the host adds the two
partials per batch.

Engine plan (per core):
  - Inputs (x, Wq, Wk, Wv, Wr) are host-cast to bf16: halves the DMA and
    runs the projection matmuls at 1 cycle/row.
  - Scores per head pair run as two row-tiled concurrent matmuls (head-even
    on PE rows 0-63, head-odd on rows 64-127, outputs in different PSUM
    banks), so the K=64 contraction doesn't waste half the array.
  - exp() on ScalarE over [128, 2, 512] (both heads of a pair per key
    block), writing bf16 `at`; causal diagonal masked on VectorE.
  - attn@V with a ones-column on V so the softmax denominators fall out of
    the same matmul (output row 64); denominators inverted with the fast
    custom-DVE reciprocal (~5x faster than the iterative one) and
    broadcast to 64 rows via a tiny PE matmul.
  - Projection / output-projection work is woven into the attention loop as
    "filler" so the PE never stalls on ScalarE exp and the HAM clock gate
    stays warm.
"""

from collections import deque

import numpy as np

import concourse.bacc as bacc
import concourse.bass as bass
import concourse.mybir as mybir
import concourse.tile as tile
from concourse.bass_utils import run_bass_kernel_spmd
from concourse.dve_ops import RECIP_APPROX_FAST_CONSTS, RECIPROCAL_APPROX_FAST

HEADS = 16
HD = 64
EMB = 1024
B, S = 4, 2048
SCALE = 1.0 / 8.0
NCORES = 8
HPC = HEADS // 2          # heads per core (8)
GW = HPC * HD             # head-group width (512)

F32 = mybir.dt.float32
F32R = mybir.dt.float32r
BF16 = mybir.dt.bfloat16
EXP = mybir.ActivationFunctionType.Exp

NQC = 4                   # query chunks of 512
QW = 512                  # query chunk width
NEC = EMB // 128          # emb chunks (8)
NSB = S // 128            # seq blocks (16)


def build():
    nc = bacc.Bacc("TRN2", target_bir_lowering=False, debug=False)

    xt_d = nc.dram_tensor("xt", [EMB, S], BF16, kind="ExternalInput")
    # weights pre-swizzled on host to partition-major so every DMA is one
    # contiguous transfer
    wq_d = nc.dram_tensor("wq", [128, 4, NEC, 128], BF16, kind="ExternalInput")
    wk_d = nc.dram_tensor("wk", [128, 4, NEC, 128], BF16, kind="ExternalInput")
    wv_d = nc.dram_tensor("wv", [128, NEC, GW], BF16, kind="ExternalInput")
    wr_d = nc.dram_tensor("wr", [128, 4, EMB], BF16, kind="ExternalInput")
    # bf16 consts: [:,0:128] causal tri mask, [:,128:256] ones, [256:640] zeros
    cb_d = nc.dram_tensor("cb", [128, 640], BF16, kind="ExternalInput")
    # f32r ones row (lhsT of the reciprocal-broadcast matmul)
    cr_d = nc.dram_tensor("cr", [1, 64], F32R, kind="ExternalInput")
    y_d = nc.dram_tensor("y", [S, EMB], F32, kind="ExternalOutput")

    rc = RECIP_APPROX_FAST_CONSTS

    with tile.TileContext(nc) as tc, nc.allow_low_precision(reason="bf16 attn"):
        with (
            tc.tile_pool(name="persist", bufs=1) as pp,
            tc.tile_pool(name="outp", bufs=4) as po,
            tc.tile_pool(name="attn", bufs=3) as pa,
            tc.tile_pool(name="recp", bufs=2) as prc,
            tc.tile_pool(name="bcp", bufs=2) as pbc,
            tc.tile_pool(name="ysb", bufs=2) as pyb,
            tc.tile_pool(name="ps_sc", bufs=2, space="PSUM") as ps_sc,
            tc.tile_pool(name="ps_out", bufs=2, space="PSUM") as ps_out,
            tc.tile_pool(name="ps_wv", bufs=2, space="PSUM") as ps_wv,
        ):
            xt = pp.tile([128, NEC, S], BF16, tag="xt")
            wq = pp.tile([128, 4, NEC, 128], BF16, tag="wq")
            wk = pp.tile([128, 4, NEC, 128], BF16, tag="wk")
            wv = pp.tile([128, NEC, GW], BF16, tag="wv")
            wr = pp.tile([128, 4, EMB], BF16, tag="wr")
            kt = pp.tile([128, NQC, S], F32R, tag="kt")
            qt = pp.tile([128, NQC, NQC, QW], F32R, tag="qt")  # [*, c, hp, q]
            v = pp.tile([128, NSB, HPC, HD + 1], BF16, tag="v")
            cb = pp.tile([128, 640], BF16, tag="cb")
            cr = pp.tile([1, 64], F32R, tag="cr")
            tri = cb[:, 0:128]
            zeros = cb[:, 256:640]

            nc.sync.dma_start(cb[:], cb_d.ap())
            nc.sync.dma_start(cr[:], cr_d.ap())
            nc.sync.dma_start(wv[:], wv_d.ap())
            for e in range(NEC):
                nc.sync.dma_start(xt[:, e, :], xt_d.ap()[e * 128:(e + 1) * 128, :])
            nc.sync.dma_start(wq[:], wq_d.ap())
            nc.sync.dma_start(wk[:], wk_d.ap())
            nc.sync.dma_start(wr[:], wr_d.ap())
            # ones column of v (softmax denominator trick)
            nc.sync.dma_start(v[:, :, :, HD], cb_d.ap()[:, 128:256])

            # HAM warmup: ~7us of throwaway matmuls on the consts tile keep
            # the PE busy through the xt DMA window, so the clock gate opens
            # (K=8/8) before the real work streams in.
            for wi in range(32):
                wps = ps_sc.tile([128, 2, QW], F32, tag="sc", name=f"warm{wi}")
                nc.tensor.matmul(wps[:, 0, :], cb[:, 0:128], cb[:, 128:640],
                                 start=True, stop=True)

            # ---------------- weave units ----------------
            def p1_unit(sb, cp):
                ps = ps_wv.tile([128, GW], F32, tag="wv", name=f"p1_{sb}")
                for e in range(NEC):
                    nc.tensor.matmul(
                        ps[:], xt[:, e, sb * 128:(sb + 1) * 128], wv[:, e, :],
                        start=(e == 0), stop=(e == NEC - 1),
                    )
                cp(
                    v[:, sb, :, 0:HD],
                    ps[:].rearrange("p (h d) -> p h d", d=HD),
                )

            def p2_unit(c, hp, is_q, cp):
                w = wq if is_q else wk
                ps = ps_wv.tile([128, QW], F32, tag="wv",
                                name=f"p2_{c}_{hp}_{int(is_q)}")
                for e in range(NEC):
                    nc.tensor.matmul(
                        ps[:], w[:, hp, e, :], xt[:, e, c * QW:(c + 1) * QW],
                        start=(e == 0), stop=(e == NEC - 1),
                    )
                if is_q:
                    cp(qt[:, c, hp, :], ps[:])
                else:
                    cp(kt[:, hp, c * QW:(c + 1) * QW], ps[:])

            def p4_unit(outtc, qc, sbl, cp, cp2=None):
                sb = qc * 4 + sbl
                ysb = pyb.tile([128, EMB], F32, tag="ysb", name=f"ysb_{sb}")
                for ncol in range(2):
                    ps = ps_wv.tile([128, QW], F32, tag="wv",
                                    name=f"p4_{sb}_{ncol}")
                    for hp in range(4):
                        nc.tensor.matmul(
                            ps[:],
                            outtc[:, hp, sbl * 128:(sbl + 1) * 128],
                            wr[:, hp, ncol * QW:(ncol + 1) * QW],
                            start=(hp == 0), stop=(hp == 3),
                        )
                    (cp2 if (cp2 and ncol) else cp)(
                        ysb[:, ncol * QW:(ncol + 1) * QW], ps[:])
                    nc.sync.dma_start(
                        y_d.ap()[sb * 128:(sb + 1) * 128,
                                 ncol * QW:(ncol + 1) * QW],
                        ysb[:, ncol * QW:(ncol + 1) * QW])

            # ---------------- prefix: v sb0-3, q/k chunk 0 ----------------
            # (ScalarE is idle here, so PSUM->SBUF copies go to it)
            for sb in range(4):
                p1_unit(sb, nc.scalar.copy)
            for hp in range(4):
                p2_unit(0, hp, True, nc.scalar.copy)
                p2_unit(0, hp, False, nc.scalar.copy)

            # Filler work woven into the attention loop so the PE never
            # stalls on ScalarE exp (VectorE copies there -- ScalarE is
            # saturated).  Emission must respect program-order dep tracking:
            #   bulk:  units attention(qc) reads from its very first
            #          iteration (q projections; chunk-1 v/k blocks whose
            #          in-chunk window is too tight) -- deadline-fenced,
            #          force-emitted before attention(qc) starts.
            #   inqc[qc]: v blocks 4qc.. and k chunk qc, consumed only from
            #          the diagonal region (kb >= 4qc) onward -- popped one
            #          per odd iteration END inside qc itself, ordered so
            #          each unit lands before its first reader.
            #   p4q:   output projections, reserved as late filler.
            bulk = deque()
            vec = nc.vector.tensor_copy
            bulk.append((1, lambda: p2_unit(1, 0, False, vec)))
            for sb in range(4, 8):
                bulk.append((1, lambda sb=sb: p1_unit(sb, vec)))
            for c in range(1, NQC):
                bulk.append((c, lambda c=c: p2_unit(c, 0, True, vec)))
            inqc = {qc: deque() for qc in range(NQC)}
            for hp in range(1, 4):
                inqc[1].append(lambda hp=hp: p2_unit(1, hp, True, vec))
            for hp in range(1, 4):
                inqc[1].append(lambda hp=hp: p2_unit(1, hp, False, vec))
            for c in (2, 3):
                for hp in range(1, 4):
                    inqc[c].append(
                        lambda c=c, hp=hp: p2_unit(c, hp, True, vec))
                inqc[c].append(lambda c=c: p2_unit(c, 0, False, vec))
                for sb in range(4 * c, 4 * c + 4):
                    inqc[c].append(lambda sb=sb: p1_unit(sb, vec))
                for hp in range(1, 4):
                    inqc[c].append(
                        lambda c=c, hp=hp: p2_unit(c, hp, False, vec))
            p4q = deque()

            def pop_filler(qc):
                if inqc[qc]:
                    inqc[qc].popleft()()
                elif bulk:
                    bulk.popleft()[1]()
                elif p4q:
                    p4q.popleft()()

            def drain_due(qc):
                while bulk and bulk[0][0] <= qc:
                    bulk.popleft()[1]()

            # ---------------- attention + output projection ----------------
            for qc in range(NQC):
                drain_due(qc)
                kbmax = 4 * (qc + 1)
                outtc = po.tile([128, NQC, QW], BF16, tag="outt",
                                name=f"outt_{qc}")
                for hp in range(4):
                    outps = [
                        ps_out.tile([HD + 1, QW], F32, tag="out",
                                    name=f"o_{qc}_{hp}_{s}")
                        for s in range(2)
                    ]
                    # Software-pipelined: score matmuls run one iteration
                    # ahead of attn@V in the PE queue, so the PE always has
                    # runway while exp(kb) is still on ScalarE.
                    atq = deque()
                    for kb in range(kbmax + 1):
                        if kb < kbmax:
                            sc = ps_sc.tile([128, 2, QW], F32, tag="sc",
                                            name=f"sc_{qc}_{hp}_{kb}")
                            for s_ in range(2):
                                ho = s_ * HD
                                nc.tensor.matmul(
                                    sc[:, s_, :],
                                    kt[ho:ho + HD, hp,
                                       kb * 128:(kb + 1) * 128],
                                    qt[ho:ho + HD, qc, hp, :],
                                    start=True, stop=True,
                                )
                            at = pa.tile([128, 2, QW], BF16, tag="at",
                                         name=f"at_{qc}_{hp}_{kb}")
                            nc.scalar.activation(at[:], sc[:], EXP)
                            j = kb - 4 * qc
                            if j >= 0:  # diagonal block: causal mask
                                for s_ in range(2):
                                    if j > 0:
                                        nc.vector.tensor_copy(
                                            at[:, s_, 0:j * 128],
                                            zeros[:, 0:j * 128])
                                    nc.vector.tensor_mul(
                                        at[:, s_, j * 128:(j + 1) * 128],
                                        at[:, s_, j * 128:(j + 1) * 128],
                                        tri,
                                    )
                            atq.append(at)
                        if kb >= 1:
                            kbp = kb - 1
                            atp = atq.popleft()
                            for s_ in range(2):
                                nc.tensor.matmul(
                                    outps[s_][:],
                                    v[:, kbp, 2 * hp + s_, :],
                                    atp[:, s_, :],
                                    start=(kbp == 0),
                                    stop=(kbp == kbmax - 1),
                                )
                            if inqc[qc]:
                                pop_filler(qc)
                            elif (kbp % 2 == 1) if bulk else (kbp % 4 == 3):
                                pop_filler(qc)

                    # epilogue: rows 0..63 = (attn@v).T numerator, row 64
                    # = softmax denominator.  Reciprocals for both heads run
                    # first; a filler chain then hides their DVE latency so
                    # the PE reaches the broadcast matmuls with inputs ready.
                    recs = []
                    for s_ in range(2):
                        den = prc.tile([1, QW], F32, tag="den",
                                       name=f"den_{qc}_{hp}_{s_}")
                        nc.vector.tensor_copy(den[0:1, :],
                                              outps[s_][HD:HD + 1, :])
                        rec = prc.tile([1, QW], F32R, tag="rec",
                                       name=f"rec_{qc}_{hp}_{s_}")
                        nc.vector._custom_dve(
                            RECIPROCAL_APPROX_FAST,
                            out=rec[0:1, :],
                            in0=den[0:1, :],
                            s0=rc["s0"], s1=rc["s1"], imm2=rc["imm2"],
                        )
                        recs.append(rec)
                    pop_filler(qc)
                    for s_ in range(2):
                        ho = s_ * HD
                        bct = ps_wv.tile([HD, QW], F32, tag="wv",
                                         name=f"bct_{qc}_{hp}_{s_}")
                        nc.tensor.matmul(bct[:], cr[0:1, 0:HD],
                                         recs[s_][0:1, :],
                                         start=True, stop=True)
                        bc = pbc.tile([HD, QW], F32R, tag="bc",
                                      name=f"bc_{qc}_{hp}_{s_}")
                        nc.vector.tensor_copy(bc[:], bct[:])
                        nc.vector.tensor_mul(
                            outtc[ho:ho + HD, hp, :], outps[s_][0:HD, :],
                            bc[:],
                        )

                # P4 of this chunk becomes filler for later chunks
                for sbl in range(4):
                    p4q.append(
                        lambda outtc=outtc, qc=qc, sbl=sbl:
                            p4_unit(outtc, qc, sbl, nc.vector.tensor_copy))

            while bulk or p4q or any(inqc.values()):
                pop_filler(NQC - 1)

    nc.compile()
    return nc


_NC_CACHE = None


def _get_nc():
    global _NC_CACHE
    if _NC_CACHE is None:
        _NC_CACHE = build()
    return _NC_CACHE


def make_in_maps(x, Wq, Wk, Wv, Wr):
    import ml_dtypes
    bf16 = ml_dtypes.bfloat16

    x = np.ascontiguousarray(x, dtype=np.float32)
    Wq = np.asarray(Wq, dtype=np.float32)
    Wk = np.asarray(Wk, dtype=np.float32)
    Wv = np.asarray(Wv, dtype=np.float32)
    Wr = np.asarray(Wr, dtype=np.float32)

    cb = np.zeros((128, 640), dtype=np.float32)
    cb[:, 0:128] = np.triu(np.ones((128, 128), dtype=np.float32))
    cb[:, 128:256] = 1.0
    cb = cb.astype(bf16)
    cr = np.ones((1, 64), dtype=np.float32)

    def swz(w):  # [1024, 512] -> [p, hp, e, n]
        return np.ascontiguousarray(
            w.reshape(NEC, 128, 4, 128).transpose(1, 2, 0, 3).astype(bf16))

    in_maps = []
    for core in range(NCORES):
        b, g = divmod(core, 2)
        hs = slice(g * GW, (g + 1) * GW)
        in_maps.append({
            "xt": np.ascontiguousarray(x[b].T.astype(bf16)),
            "wq": swz(Wq[:, hs] * SCALE),
            "wk": swz(Wk[:, hs]),
            "wv": np.ascontiguousarray(
                Wv[:, hs].reshape(NEC, 128, GW).transpose(1, 0, 2).astype(bf16)),
            "wr": np.ascontiguousarray(
                Wr[hs, :].reshape(4, 128, EMB).transpose(1, 0, 2).astype(bf16)),
            "cb": cb,
            "cr": cr,
        })
    return in_maps


def kernel(x, Wq, Wk, Wv, Wr):
    in_maps = make_in_maps(x, Wq, Wk, Wv, Wr)
    nc = _get_nc()
    res = run_bass_kernel_spmd(nc, in_maps, core_ids=list(range(NCORES)))

    y = np.empty((B, S, EMB), dtype=np.float32)
    for b in range(B):
        y[b] = res.results[2 * b]["y"] + res.results[2 * b + 1]["y"]
    return y


# revision 35
# speedup vs baseline: 1.2300x; 1.2300x over previous
"""Causal multi-head attention (B=4, S=2048, H=16, D=64, E=1024) on 8 TRN2 cores.

Sharding: data-parallel over batch (4) x tensor-parallel over heads (2 groups
of 8). Each core computes, for its (batch, head-group):
    q/k/v projections -> causal softmax attention -> output projection
and returns a partial [S, E] output (Wr row-split); the host adds the two
partials per batch.

Engine plan (per core):
  - Inputs (x, Wq, Wk, Wv, Wr) are host-cast to bf16: halves the DMA and
    runs the projection matmuls at 1 cycle/row.
  - Scores per head pair run as two row-tiled concurrent matmuls (head-even
    on PE rows 0-63, head-odd on rows 64-127, outputs in different PSUM
    banks), so the K=64 contraction doesn't waste half the array.
  - exp() on ScalarE over [128, 2, 512] (both heads of a pair per key
    block), writing bf16 `at`; causal diagonal masked on VectorE.
  - attn@V with a ones-column on V so the softmax denominators fall out of
    the same matmul (output row 64); denominators inverted with the fast
    custom-DVE reciprocal (~5x faster than the iterative one) and
    broadcast to 64 rows via a tiny PE matmul.
  - Projection / output-projection work is woven into the attention loop as
    "filler" so the PE never stalls on ScalarE exp and the HAM clock gate
    stays warm.
"""

from collections import deque

import numpy as np

import concourse.bacc as bacc
import concourse.bass as bass
import concourse.mybir as mybir
import concourse.tile as tile
from concourse.bass_utils import run_bass_kernel_spmd
from concourse.dve_ops import RECIP_APPROX_FAST_CONSTS, RECIPROCAL_APPROX_FAST

HEADS = 16
HD = 64
EMB = 1024
B, S = 4, 2048
SCALE = 1.0 / 8.0
NCORES = 8
HPC = HEADS // 2          # heads per core (8)
GW = HPC * HD             # head-group width (512)

F32 = mybir.dt.float32
F32R = mybir.dt.float32r
BF16 = mybir.dt.bfloat16
EXP = mybir.ActivationFunctionType.Exp

NQC = 4                   # query chunks of 512
QW = 512                  # query chunk width
NEC = EMB // 128          # emb chunks (8)
NSB = S // 128            # seq blocks (16)


def build():
    nc = bacc.Bacc("TRN2", target_bir_lowering=False, debug=False)

    xt_d = nc.dram_tensor("xt", [EMB, S], BF16, kind="ExternalInput")
    # weights pre-swizzled on host to partition-major so every DMA is one
    # contiguous transfer
    wq_d = nc.dram_tensor("wq", [128, 4, NEC, 128], BF16, kind="ExternalInput")
    wk_d = nc.dram_tensor("wk", [128, 4, NEC, 128], BF16, kind="ExternalInput")
    wv_d = nc.dram_tensor("wv", [128, NEC, GW], BF16, kind="ExternalInput")
    wr_d = nc.dram_tensor("wr", [128, 4, EMB], BF16, kind="ExternalInput")
    # bf16 consts: [:,0:128] causal tri mask, [:,128:256] ones, [256:640] zeros
    cb_d = nc.dram_tensor("cb", [128, 640], BF16, kind="ExternalInput")
    # f32r ones row (lhsT of the reciprocal-broadcast matmul)
    cr_d = nc.dram_tensor("cr", [1, 64], F32R, kind="ExternalInput")
    y_d = nc.dram_tensor("y", [S, EMB], F32, kind="ExternalOutput")

    rc = RECIP_APPROX_FAST_CONSTS

    with tile.TileContext(nc) as tc, nc.allow_low_precision(reason="bf16 attn"):
        with (
            tc.tile_pool(name="persist", bufs=1) as pp,
            tc.tile_pool(name="outp", bufs=4) as po,
            tc.tile_pool(name="attn", bufs=3) as pa,
            tc.tile_pool(name="recp", bufs=2) as prc,
            tc.tile_pool(name="bcp", bufs=2) as pbc,
            tc.tile_pool(name="ysb", bufs=2) as pyb,
            tc.tile_pool(name="ps_sc", bufs=2, space="PSUM") as ps_sc,
            tc.tile_pool(name="ps_out", bufs=2, space="PSUM") as ps_out,
            tc.tile_pool(name="ps_wv", bufs=2, space="PSUM") as ps_wv,
        ):
            xt = pp.tile([128, NEC, S], BF16, tag="xt")
            wq = pp.tile([128, 4, NEC, 128], BF16, tag="wq")
            wk = pp.tile([128, 4, NEC, 128], BF16, tag="wk")
            wv = pp.tile([128, NEC, GW], BF16, tag="wv")
            wr = pp.tile([128, 4, EMB], BF16, tag="wr")
            kt = pp.tile([128, NQC, S], F32R, tag="kt")
            qt = pp.tile([128, NQC, NQC, QW], F32R, tag="qt")  # [*, c, hp, q]
            v = pp.tile([128, NSB, HPC, HD + 1], BF16, tag="v")
            cb = pp.tile([128, 640], BF16, tag="cb")
            cr = pp.tile([1, 64], F32R, tag="cr")
            tri = cb[:, 0:128]
            zeros = cb[:, 256:640]

            nc.sync.dma_start(cb[:], cb_d.ap())
            nc.sync.dma_start(cr[:], cr_d.ap())
            nc.sync.dma_start(wv[:], wv_d.ap())
            for e in range(NEC):
                nc.sync.dma_start(xt[:, e, :], xt_d.ap()[e * 128:(e + 1) * 128, :])
            nc.sync.dma_start(wq[:], wq_d.ap())
            nc.sync.dma_start(wk[:], wk_d.ap())
            nc.sync.dma_start(wr[:], wr_d.ap())
            # ones column of v (softmax denominator trick)
            nc.sync.dma_start(v[:, :, :, HD], cb_d.ap()[:, 128:256])

            # HAM warmup: ~7us of throwaway matmuls on the consts tile keep
            # the PE busy through the xt DMA window, so the clock gate opens
            # (K=8/8) before the real work streams in.
            for wi in range(32):
                wps = ps_sc.tile([128, 2, QW], F32, tag="sc", name=f"warm{wi}")
                nc.tensor.matmul(wps[:, 0, :], cb[:, 0:128], cb[:, 128:640],
                                 start=True, stop=True)

            # ---------------- weave units ----------------
            def p1_unit(sb, cp):
                ps = ps_wv.tile([128, GW], F32, tag="wv", name=f"p1_{sb}")
                for e in range(NEC):
                    nc.tensor.matmul(
                        ps[:], xt[:, e, sb * 128:(sb + 1) * 128], wv[:, e, :],
                        start=(e == 0), stop=(e == NEC - 1),
                    )
                cp(
                    v[:, sb, :, 0:HD],
                    ps[:].rearrange("p (h d) -> p h d", d=HD),
                )

            def p2_unit(c, hp, is_q, cp):
                w = wq if is_q else wk
                ps = ps_wv.tile([128, QW], F32, tag="wv",
                                name=f"p2_{c}_{hp}_{int(is_q)}")
                for e in range(NEC):
                    nc.tensor.matmul(
                        ps[:], w[:, hp, e, :], xt[:, e, c * QW:(c + 1) * QW],
                        start=(e == 0), stop=(e == NEC - 1),
                    )
                if is_q:
                    cp(qt[:, c, hp, :], ps[:])
                else:
                    cp(kt[:, hp, c * QW:(c + 1) * QW], ps[:])

            def p4_unit(outtc, qc, sbl, cp, cp2=None):
                sb = qc * 4 + sbl
                ysb = pyb.tile([128, EMB], F32, tag="ysb", name=f"ysb_{sb}")
                for ncol in range(2):
                    ps = ps_wv.tile([128, QW], F32, tag="wv",
                                    name=f"p4_{sb}_{ncol}")
                    for hp in range(4):
                        nc.tensor.matmul(
                            ps[:],
                            outtc[:, hp, sbl * 128:(sbl + 1) * 128],
                            wr[:, hp, ncol * QW:(ncol + 1) * QW],
                            start=(hp == 0), stop=(hp == 3),
                        )
                    (cp2 if (cp2 and ncol) else cp)(
                        ysb[:, ncol * QW:(ncol + 1) * QW], ps[:])
                    nc.sync.dma_start(
                        y_d.ap()[sb * 128:(sb + 1) * 128,
                                 ncol * QW:(ncol + 1) * QW],
                        ysb[:, ncol * QW:(ncol + 1) * QW])

            # ---------------- prefix: v sb0-3, q/k chunk 0 ----------------
            # (ScalarE is idle here, so PSUM->SBUF copies go to it)
            for sb in range(4):
                p1_unit(sb, nc.scalar.copy)
            for hp in range(4):
                p2_unit(0, hp, True, nc.scalar.copy)
                p2_unit(0, hp, False, nc.scalar.copy)

            # Filler work woven into the attention loop so the PE never
            # stalls on ScalarE exp (VectorE copies there -- ScalarE is
            # saturated).  Emission must respect program-order dep tracking:
            #   bulk:  units attention(qc) reads from its very first
            #          iteration (q projections; chunk-1 v/k blocks whose
            #          in-chunk window is too tight) -- deadline-fenced,
            #          force-emitted before attention(qc) starts.
            #   inqc[qc]: v blocks 4qc.. and k chunk qc, consumed only from
            #          the diagonal region (kb >= 4qc) onward -- popped one
            #          per odd iteration END inside qc itself, ordered so
            #          each unit lands before its first reader.
            #   p4q:   output projections, reserved as late filler.
            bulk = deque()
            vec = nc.vector.tensor_copy
            bulk.append((1, lambda: p2_unit(1, 0, False, vec)))
            for sb in range(4, 8):
                bulk.append((1, lambda sb=sb: p1_unit(sb, vec)))
            for c in range(1, NQC):
                for hp in range(4):
                    bulk.append((c, lambda c=c, hp=hp: p2_unit(
                        c, hp, True, vec)))
            inqc = {qc: deque() for qc in range(NQC)}
            for hp in range(1, 4):
                inqc[1].append(lambda hp=hp: p2_unit(1, hp, False, vec))
            for c in (2, 3):
                inqc[c].append(lambda c=c: p2_unit(c, 0, False, vec))
                for sb in range(4 * c, 4 * c + 4):
                    inqc[c].append(lambda sb=sb: p1_unit(sb, vec))
                for hp in range(1, 4):
                    inqc[c].append(
                        lambda c=c, hp=hp: p2_unit(c, hp, False, vec))
            p4q = deque()

            def pop_filler(qc):
                if inqc[qc]:
                    inqc[qc].popleft()()
                elif bulk:
                    bulk.popleft()[1]()
                elif p4q:
                    p4q.popleft()()

            def drain_due(qc):
                while bulk and bulk[0][0] <= qc:
                    bulk.popleft()[1]()

            # ---------------- attention + output projection ----------------
            for qc in range(NQC):
                drain_due(qc)
                kbmax = 4 * (qc + 1)
                outtc = po.tile([128, NQC, QW], BF16, tag="outt",
                                name=f"outt_{qc}")
                for hp in range(4):
                    outps = [
                        ps_out.tile([HD + 1, QW], F32, tag="out",
                                    name=f"o_{qc}_{hp}_{s}")
                        for s in range(2)
                    ]
                    # Software-pipelined: score matmuls run one iteration
                    # ahead of attn@V in the PE queue, so the PE always has
                    # runway while exp(kb) is still on ScalarE.
                    atq = deque()
                    for kb in range(kbmax + 1):
                        if kb < kbmax:
                            sc = ps_sc.tile([128, 2, QW], F32, tag="sc",
                                            name=f"sc_{qc}_{hp}_{kb}")
                            for s_ in range(2):
                                ho = s_ * HD
                                nc.tensor.matmul(
                                    sc[:, s_, :],
                                    kt[ho:ho + HD, hp,
                                       kb * 128:(kb + 1) * 128],
                                    qt[ho:ho + HD, qc, hp, :],
                                    start=True, stop=True,
                                )
                            at = pa.tile([128, 2, QW], BF16, tag="at",
                                         name=f"at_{qc}_{hp}_{kb}")
                            nc.scalar.activation(at[:], sc[:], EXP)
                            j = kb - 4 * qc
                            if j >= 0:  # diagonal block: causal mask
                                for s_ in range(2):
                                    if j > 0:
                                        nc.vector.tensor_copy(
                                            at[:, s_, 0:j * 128],
                                            zeros[:, 0:j * 128])
                                    nc.vector.tensor_mul(
                                        at[:, s_, j * 128:(j + 1) * 128],
                                        at[:, s_, j * 128:(j + 1) * 128],
                                        tri,
                                    )
                            atq.append(at)
                        if kb >= 1:
                            kbp = kb - 1
                            atp = atq.popleft()
                            for s_ in range(2):
                                nc.tensor.matmul(
                                    outps[s_][:],
                                    v[:, kbp, 2 * hp + s_, :],
                                    atp[:, s_, :],
                                    start=(kbp == 0),
                                    stop=(kbp == kbmax - 1),
                                )
                            busy_q = inqc[qc] or bulk
                            if (kbp % 2 == 1) if busy_q else (kbp % 4 == 3):
                                pop_filler(qc)

                    # epilogue: rows 0..63 = (attn@v).T numerator, row 64
                    # = softmax denominator.  Reciprocals for both heads run
                    # first; a filler chain then hides their DVE latency so
                    # the PE reaches the broadcast matmuls with inputs ready.
                    recs = []
                    for s_ in range(2):
                        den = prc.tile([1, QW], F32, tag="den",
                                       name=f"den_{qc}_{hp}_{s_}")
                        nc.vector.tensor_copy(den[0:1, :],
                                              outps[s_][HD:HD + 1, :])
                        rec = prc.tile([1, QW], F32R, tag="rec",
                                       name=f"rec_{qc}_{hp}_{s_}")
                        nc.vector._custom_dve(
                            RECIPROCAL_APPROX_FAST,
                            out=rec[0:1, :],
                            in0=den[0:1, :],
                            s0=rc["s0"], s1=rc["s1"], imm2=rc["imm2"],
                        )
                        recs.append(rec)
                    pop_filler(qc)
                    for s_ in range(2):
                        ho = s_ * HD
                        bct = ps_wv.tile([HD, QW], F32, tag="wv",
                                         name=f"bct_{qc}_{hp}_{s_}")
                        nc.tensor.matmul(bct[:], cr[0:1, 0:HD],
                                         recs[s_][0:1, :],
                                         start=True, stop=True)
                        bc = pbc.tile([HD, QW], F32R, tag="bc",
                                      name=f"bc_{qc}_{hp}_{s_}")
                        nc.vector.tensor_copy(bc[:], bct[:])
                        nc.vector.tensor_mul(
                            outtc[ho:ho + HD, hp, :], outps[s_][0:HD, :],
                            bc[:],
                        )

                # P4 of this chunk becomes filler for later chunks
                for sbl in range(4):
                    p4q.append(
                        lambda outtc=outtc, qc=qc, sbl=sbl:
                            p4_unit(outtc, qc, sbl, nc.vector.tensor_copy))

            while bulk or p4q or any(inqc.values()):
                pop_filler(NQC - 1)

    nc.compile()
    return nc


_NC_CACHE = None


def _get_nc():
    global _NC_CACHE
    if _NC_CACHE is None:
        _NC_CACHE = build()
    return _NC_CACHE


def make_in_maps(x, Wq, Wk, Wv, Wr):
    import ml_dtypes
    bf16 = ml_dtypes.bfloat16

    x = np.ascontiguousarray(x, dtype=np.float32)
    Wq = np.asarray(Wq, dtype=np.float32)
    Wk = np.asarray(Wk, dtype=np.float32)
    Wv = np.asarray(Wv, dtype=np.float32)
    Wr = np.asarray(Wr, dtype=np.float32)

    cb = np.zeros((128, 640), dtype=np.float32)
    cb[:, 0:128] = np.triu(np.ones((128, 128), dtype=np.float32))
    cb[:, 128:256] = 1.0
    cb = cb.astype(bf16)
    cr = np.ones((1, 64), dtype=np.float32)

    def swz(w):  # [1024, 512] -> [p, hp, e, n]
        return np.ascontiguousarray(
            w.reshape(NEC, 128, 4, 128).transpose(1, 2, 0, 3).astype(bf16))

    in_maps = []
    for core in range(NCORES):
        b, g = divmod(core, 2)
        hs = slice(g * GW, (g + 1) * GW)
        in_maps.append({
            "xt": np.ascontiguousarray(x[b].T.astype(bf16)),
            "wq": swz(Wq[:, hs] * SCALE),
            "wk": swz(Wk[:, hs]),
            "wv": np.ascontiguousarray(
                Wv[:, hs].reshape(NEC, 128, GW).transpose(1, 0, 2).astype(bf16)),
            "wr": np.ascontiguousarray(
                Wr[hs, :].reshape(4, 128, EMB).transpose(1, 0, 2).astype(bf16)),
            "cb": cb,
            "cr": cr,
        })
    return in_maps


def kernel(x, Wq, Wk, Wv, Wr):
    in_maps = make_in_maps(x, Wq, Wk, Wv, Wr)
    nc = _get_nc()
    res = run_bass_kernel_spmd(nc, in_maps, core_ids=list(range(NCORES)))

    y = np.empty((B, S, EMB), dtype=np.float32)
    for b in range(B):
        y[b] = res.results[2 * b]["y"] + res.results[2 * b + 1]["y"]
    return y


# revision 36
# speedup vs baseline: 1.2819x; 1.0422x over previous
"""Causal multi-head attention (B=4, S=2048, H=16, D=64, E=1024) on 8 TRN2 cores.

Sharding: data-parallel over batch (4) x tensor-parallel over heads (2 groups
of 8). Each core computes, for its (batch, head-group):
    q/k/v projections -> causal softmax attention -> output projection
and returns a partial [S, E] output (Wr row-split); the host adds the two
partials per batch.

Engine plan (per core):
  - Inputs (x, Wq, Wk, Wv, Wr) are host-cast to bf16: halves the DMA and
    runs the projection matmuls at 1 cycle/row.
  - Scores per head pair run as two row-tiled concurrent matmuls (head-even
    on PE rows 0-63, head-odd on rows 64-127, outputs in different PSUM
    banks), so the K=64 contraction doesn't waste half the array.
  - exp() on ScalarE over [128, 2, 512] (both heads of a pair per key
    block), writing bf16 `at`; causal diagonal masked on VectorE.
  - attn@V with a ones-column on V so the softmax denominators fall out of
    the same matmul (output row 64); denominators inverted with the fast
    custom-DVE reciprocal (~5x faster than the iterative one) and
    broadcast to 64 rows via a tiny PE matmul.
  - Projection / output-projection work is woven into the attention loop as
    "filler" so the PE never stalls on ScalarE exp and the HAM clock gate
    stays warm.
"""

from collections import deque

import numpy as np

import concourse.bacc as bacc
import concourse.bass as bass
import concourse.mybir as mybir
import concourse.tile as tile
from concourse.bass_utils import run_bass_kernel_spmd
from concourse.dve_ops import RECIP_APPROX_FAST_CONSTS, RECIPROCAL_APPROX_FAST

HEADS = 16
HD = 64
EMB = 1024
B, S = 4, 2048
SCALE = 1.0 / 8.0
NCORES = 8
HPC = HEADS // 2          # heads per core (8)
GW = HPC * HD             # head-group width (512)

F32 = mybir.dt.float32
F32R = mybir.dt.float32r
BF16 = mybir.dt.bfloat16
EXP = mybir.ActivationFunctionType.Exp

NQC = 4                   # query chunks of 512
QW = 512                  # query chunk width
NEC = EMB // 128          # emb chunks (8)
NSB = S // 128            # seq blocks (16)


def build():
    nc = bacc.Bacc("TRN2", target_bir_lowering=False, debug=False)

    xt_d = nc.dram_tensor("xt", [EMB, S], BF16, kind="ExternalInput")
    # weights pre-swizzled on host to partition-major so every DMA is one
    # contiguous transfer
    wq_d = nc.dram_tensor("wq", [128, 4, NEC, 128], BF16, kind="ExternalInput")
    wk_d = nc.dram_tensor("wk", [128, 4, NEC, 128], BF16, kind="ExternalInput")
    wv_d = nc.dram_tensor("wv", [128, NEC, GW], BF16, kind="ExternalInput")
    wr_d = nc.dram_tensor("wr", [128, 4, EMB], BF16, kind="ExternalInput")
    # bf16 consts: [:,0:128] causal tri mask, [:,128:256] ones, [256:640] zeros
    cb_d = nc.dram_tensor("cb", [128, 640], BF16, kind="ExternalInput")
    # f32r ones row (lhsT of the reciprocal-broadcast matmul)
    cr_d = nc.dram_tensor("cr", [1, 64], F32R, kind="ExternalInput")
    y_d = nc.dram_tensor("y", [S, EMB], F32, kind="ExternalOutput")

    rc = RECIP_APPROX_FAST_CONSTS

    with tile.TileContext(nc) as tc, nc.allow_low_precision(reason="bf16 attn"):
        with (
            tc.tile_pool(name="persist", bufs=1) as pp,
            tc.tile_pool(name="outp", bufs=4) as po,
            tc.tile_pool(name="attn", bufs=4) as pa,
            tc.tile_pool(name="recp", bufs=2) as prc,
            tc.tile_pool(name="bcp", bufs=2) as pbc,
            tc.tile_pool(name="ysb", bufs=3) as pyb,
            tc.tile_pool(name="ps_sc", bufs=2, space="PSUM") as ps_sc,
            tc.tile_pool(name="ps_out", bufs=2, space="PSUM") as ps_out,
            tc.tile_pool(name="ps_wv", bufs=2, space="PSUM") as ps_wv,
        ):
            xt = pp.tile([128, NEC, S], BF16, tag="xt")
            wq = pp.tile([128, 4, NEC, 128], BF16, tag="wq")
            wk = pp.tile([128, 4, NEC, 128], BF16, tag="wk")
            wv = pp.tile([128, NEC, GW], BF16, tag="wv")
            wr = pp.tile([128, 4, EMB], BF16, tag="wr")
            kt = pp.tile([128, NQC, S], F32R, tag="kt")
            qt = pp.tile([128, NQC, NQC, QW], F32R, tag="qt")  # [*, c, hp, q]
            v = pp.tile([128, NSB, HPC, HD + 1], BF16, tag="v")
            cb = pp.tile([128, 640], BF16, tag="cb")
            cr = pp.tile([1, 64], F32R, tag="cr")
            tri = cb[:, 0:128]
            zeros = cb[:, 256:640]

            nc.sync.dma_start(cb[:], cb_d.ap())
            nc.sync.dma_start(cr[:], cr_d.ap())
            nc.sync.dma_start(wv[:], wv_d.ap())
            for e in range(NEC):
                nc.sync.dma_start(xt[:, e, :], xt_d.ap()[e * 128:(e + 1) * 128, :])
            nc.sync.dma_start(wq[:], wq_d.ap())
            nc.sync.dma_start(wk[:], wk_d.ap())
            nc.sync.dma_start(wr[:], wr_d.ap())
            # ones column of v (softmax denominator trick)
            nc.sync.dma_start(v[:, :, :, HD], cb_d.ap()[:, 128:256])

            # HAM warmup: ~7us of throwaway matmuls on the consts tile keep
            # the PE busy through the xt DMA window, so the clock gate opens
            # (K=8/8) before the real work streams in.
            for wi in range(32):
                wps = ps_sc.tile([128, 2, QW], F32, tag="sc", name=f"warm{wi}")
                nc.tensor.matmul(wps[:, 0, :], cb[:, 0:128], cb[:, 128:640],
                                 start=True, stop=True)

            # ---------------- weave units ----------------
            def p1_unit(sb, cp):
                ps = ps_wv.tile([128, GW], F32, tag="wv", name=f"p1_{sb}")
                for e in range(NEC):
                    nc.tensor.matmul(
                        ps[:], xt[:, e, sb * 128:(sb + 1) * 128], wv[:, e, :],
                        start=(e == 0), stop=(e == NEC - 1),
                    )
                cp(
                    v[:, sb, :, 0:HD],
                    ps[:].rearrange("p (h d) -> p h d", d=HD),
                )

            def p2_unit(c, hp, is_q, cp):
                w = wq if is_q else wk
                ps = ps_wv.tile([128, QW], F32, tag="wv",
                                name=f"p2_{c}_{hp}_{int(is_q)}")
                for e in range(NEC):
                    nc.tensor.matmul(
                        ps[:], w[:, hp, e, :], xt[:, e, c * QW:(c + 1) * QW],
                        start=(e == 0), stop=(e == NEC - 1),
                    )
                if is_q:
                    cp(qt[:, c, hp, :], ps[:])
                else:
                    cp(kt[:, hp, c * QW:(c + 1) * QW], ps[:])

            def p4_unit(outtc, qc, sbl, cp, cp2=None):
                sb = qc * 4 + sbl
                ysb = pyb.tile([128, EMB], F32, tag="ysb", name=f"ysb_{sb}")
                for ncol in range(2):
                    ps = ps_wv.tile([128, QW], F32, tag="wv",
                                    name=f"p4_{sb}_{ncol}")
                    for hp in range(4):
                        nc.tensor.matmul(
                            ps[:],
                            outtc[:, hp, sbl * 128:(sbl + 1) * 128],
                            wr[:, hp, ncol * QW:(ncol + 1) * QW],
                            start=(hp == 0), stop=(hp == 3),
                        )
                    (cp2 if (cp2 and ncol) else cp)(
                        ysb[:, ncol * QW:(ncol + 1) * QW], ps[:])
                    nc.sync.dma_start(
                        y_d.ap()[sb * 128:(sb + 1) * 128,
                                 ncol * QW:(ncol + 1) * QW],
                        ysb[:, ncol * QW:(ncol + 1) * QW])

            # ---------------- prefix: v sb0-3, q/k chunk 0 ----------------
            # (ScalarE is idle here, so PSUM->SBUF copies go to it)
            for sb in range(4):
                p1_unit(sb, nc.scalar.copy)
            for hp in range(4):
                p2_unit(0, hp, True, nc.scalar.copy)
                p2_unit(0, hp, False, nc.scalar.copy)

            # Filler work woven into the attention loop so the PE never
            # stalls on ScalarE exp (VectorE copies there -- ScalarE is
            # saturated).  Emission must respect program-order dep tracking:
            #   bulk:  units attention(qc) reads from its very first
            #          iteration (q projections; chunk-1 v/k blocks whose
            #          in-chunk window is too tight) -- deadline-fenced,
            #          force-emitted before attention(qc) starts.
            #   inqc[qc]: v blocks 4qc.. and k chunk qc, consumed only from
            #          the diagonal region (kb >= 4qc) onward -- popped one
            #          per odd iteration END inside qc itself, ordered so
            #          each unit lands before its first reader.
            #   p4q:   output projections, reserved as late filler.
            bulk = deque()
            vec = nc.vector.tensor_copy
            bulk.append((1, lambda: p2_unit(1, 0, False, vec)))
            for sb in range(4, 8):
                bulk.append((1, lambda sb=sb: p1_unit(sb, vec)))
            for c in range(1, NQC):
                for hp in range(4):
                    bulk.append((c, lambda c=c, hp=hp: p2_unit(
                        c, hp, True, vec)))
            inqc = {qc: deque() for qc in range(NQC)}
            for hp in range(1, 4):
                inqc[1].append(lambda hp=hp: p2_unit(1, hp, False, vec))
            for c in (2, 3):
                inqc[c].append(lambda c=c: p2_unit(c, 0, False, vec))
                for sb in range(4 * c, 4 * c + 4):
                    inqc[c].append(lambda sb=sb: p1_unit(sb, vec))
                for hp in range(1, 4):
                    inqc[c].append(
                        lambda c=c, hp=hp: p2_unit(c, hp, False, vec))
            p4q = deque()

            def pop_filler(qc):
                if inqc[qc]:
                    inqc[qc].popleft()()
                elif bulk:
                    bulk.popleft()[1]()
                elif p4q:
                    p4q.popleft()()

            def drain_due(qc):
                while bulk and bulk[0][0] <= qc:
                    bulk.popleft()[1]()

            # ---------------- attention + output projection ----------------
            for qc in range(NQC):
                drain_due(qc)
                kbmax = 4 * (qc + 1)
                outtc = po.tile([128, NQC, QW], BF16, tag="outt",
                                name=f"outt_{qc}")
                for hp in range(4):
                    outps = [
                        ps_out.tile([HD + 1, QW], F32, tag="out",
                                    name=f"o_{qc}_{hp}_{s}")
                        for s in range(2)
                    ]
                    # Software-pipelined: score matmuls run one iteration
                    # ahead of attn@V in the PE queue, so the PE always has
                    # runway while exp(kb) is still on ScalarE.
                    atq = deque()
                    for kb in range(kbmax + 1):
                        if kb < kbmax:
                            sc = ps_sc.tile([128, 2, QW], F32, tag="sc",
                                            name=f"sc_{qc}_{hp}_{kb}")
                            for s_ in range(2):
                                ho = s_ * HD
                                nc.tensor.matmul(
                                    sc[:, s_, :],
                                    kt[ho:ho + HD, hp,
                                       kb * 128:(kb + 1) * 128],
                                    qt[ho:ho + HD, qc, hp, :],
                                    start=True, stop=True,
                                )
                            at = pa.tile([128, 2, QW], BF16, tag="at",
                                         name=f"at_{qc}_{hp}_{kb}")
                            nc.scalar.activation(at[:], sc[:], EXP)
                            j = kb - 4 * qc
                            if j >= 0:  # diagonal block: causal mask
                                for s_ in range(2):
                                    if j > 0:
                                        nc.vector.tensor_copy(
                                            at[:, s_, 0:j * 128],
                                            zeros[:, 0:j * 128])
                                    nc.vector.tensor_mul(
                                        at[:, s_, j * 128:(j + 1) * 128],
                                        at[:, s_, j * 128:(j + 1) * 128],
                                        tri,
                                    )
                            atq.append(at)
                        if kb >= 1:
                            kbp = kb - 1
                            atp = atq.popleft()
                            for s_ in range(2):
                                nc.tensor.matmul(
                                    outps[s_][:],
                                    v[:, kbp, 2 * hp + s_, :],
                                    atp[:, s_, :],
                                    start=(kbp == 0),
                                    stop=(kbp == kbmax - 1),
                                )
                            busy_q = inqc[qc] or bulk
                            if (kbp % 2 == 1) if busy_q else (kbp % 4 == 3):
                                pop_filler(qc)

                    # epilogue: rows 0..63 = (attn@v).T numerator, row 64
                    # = softmax denominator.  Reciprocals for both heads run
                    # first; a filler chain then hides their DVE latency so
                    # the PE reaches the broadcast matmuls with inputs ready.
                    recs = []
                    for s_ in range(2):
                        den = prc.tile([1, QW], F32, tag="den",
                                       name=f"den_{qc}_{hp}_{s_}")
                        nc.vector.tensor_copy(den[0:1, :],
                                              outps[s_][HD:HD + 1, :])
                        rec = prc.tile([1, QW], F32R, tag="rec",
                                       name=f"rec_{qc}_{hp}_{s_}")
                        nc.vector._custom_dve(
                            RECIPROCAL_APPROX_FAST,
                            out=rec[0:1, :],
                            in0=den[0:1, :],
                            s0=rc["s0"], s1=rc["s1"], imm2=rc["imm2"],
                        )
                        recs.append(rec)
                    pop_filler(qc)
                    for s_ in range(2):
                        ho = s_ * HD
                        bct = ps_wv.tile([HD, QW], F32, tag="wv",
                                         name=f"bct_{qc}_{hp}_{s_}")
                        nc.tensor.matmul(bct[:], cr[0:1, 0:HD],
                                         recs[s_][0:1, :],
                                         start=True, stop=True)
                        bc = pbc.tile([HD, QW], F32R, tag="bc",
                                      name=f"bc_{qc}_{hp}_{s_}")
                        nc.vector.tensor_copy(bc[:], bct[:])
                        nc.vector.tensor_mul(
                            outtc[ho:ho + HD, hp, :], outps[s_][0:HD, :],
                            bc[:],
                        )

                # P4 of this chunk becomes filler for later chunks
                for sbl in range(4):
                    p4q.append(
                        lambda outtc=outtc, qc=qc, sbl=sbl:
                            p4_unit(outtc, qc, sbl, nc.vector.tensor_copy))

            while bulk or p4q or any(inqc.values()):
                pop_filler(NQC - 1)

    nc.compile()
    return nc


_NC_CACHE = None


def _get_nc():
    global _NC_CACHE
    if _NC_CACHE is None:
        _NC_CACHE = build()
    return _NC_CACHE


def make_in_maps(x, Wq, Wk, Wv, Wr):
    import ml_dtypes
    bf16 = ml_dtypes.bfloat16

    x = np.ascontiguousarray(x, dtype=np.float32)
    Wq = np.asarray(Wq, dtype=np.float32)
    Wk = np.asarray(Wk, dtype=np.float32)
    Wv = np.asarray(Wv, dtype=np.float32)
    Wr = np.asarray(Wr, dtype=np.float32)

    cb = np.zeros((128, 640), dtype=np.float32)
    cb[:, 0:128] = np.triu(np.ones((128, 128), dtype=np.float32))
    cb[:, 128:256] = 1.0
    cb = cb.astype(bf16)
    cr = np.ones((1, 64), dtype=np.float32)

    def swz(w):  # [1024, 512] -> [p, hp, e, n]
        return np.ascontiguousarray(
            w.reshape(NEC, 128, 4, 128).transpose(1, 2, 0, 3).astype(bf16))

    in_maps = []
    for core in range(NCORES):
        b, g = divmod(core, 2)
        hs = slice(g * GW, (g + 1) * GW)
        in_maps.append({
            "xt": np.ascontiguousarray(x[b].T.astype(bf16)),
            "wq": swz(Wq[:, hs] * SCALE),
            "wk": swz(Wk[:, hs]),
            "wv": np.ascontiguousarray(
                Wv[:, hs].reshape(NEC, 128, GW).transpose(1, 0, 2).astype(bf16)),
            "wr": np.ascontiguousarray(
                Wr[hs, :].reshape(4, 128, EMB).transpose(1, 0, 2).astype(bf16)),
            "cb": cb,
            "cr": cr,
        })
    return in_maps


def kernel(x, Wq, Wk, Wv, Wr):
    in_maps = make_in_maps(x, Wq, Wk, Wv, Wr)
    nc = _get_nc()
    res = run_bass_kernel_spmd(nc, in_maps, core_ids=list(range(NCORES)))

    y = np.empty((B, S, EMB), dtype=np.float32)
    for b in range(B):
        y[b] = res.results[2 * b]["y"] + res.results[2 * b + 1]["y"]
    return y
